# revision 38
# baseline (speedup 1.0000x reference)
"""Multi-head attention (B=4, S=2048, D=1024, H=16, dk=dv=64, causal + key/query
masks) on 8 Trainium2 NeuronCores via Bass/Tile.

Sharding: core = (batch, query-half). Each core handles one batch's 1024 query
rows (two 512-row chunks paired across the causal diagonal: {0,3} or {1,2}) and
redundantly computes K/V projections for its batch — no collectives.

v2 structure (vs baseline): the attention inner loop processes k-tile PAIRS —
scores for two 128-key tiles land in one [128,2,512] PSUM tile and a single
activation computes exp over 1024 free elements, halving ACT-engine instruction
count and amortizing its fixed access latency. v_mask is folded into the V
tiles at projection-copy time (rows zeroed, denominator column = vm), so the
activation needs no per-key bias; a per-pair bias (-1e12) zeroes the dummy
pairs that pad the causal structure. Fully-masked query rows (reference
softmax = uniform over the exactly -1e12 entries) are reproduced with the
raw (unmasked) k-tile-0 V tile plus two accumulating fix matmuls, emitted only
for the qb=0 slot (fix rows are always < 128). All PSUM->SBUF copies run on
DVE/Pool so the ACT engine does exp exclusively.
"""

import os

os.environ.setdefault("JAX_PLATFORMS", "axon,cpu")

import numpy as np
import ml_dtypes
from contextlib import ExitStack

import concourse.bass as bass
import concourse.mybir as mybir
import concourse.tile as tile
from concourse import bacc
from concourse import bass_utils

F32 = mybir.dt.float32
FP8 = mybir.dt.float8e4
F32R = mybir.dt.float32r
BF16 = mybir.dt.bfloat16
AF = mybir.ActivationFunctionType

B, S, D = 4, 2048, 1024
H, DK, DV = 16, 64, 64
NCORES = 8
QB = 512          # query block (matmul moving dim)
KT = 128          # key tile (scores partition dim)
NKT = S // KT     # 16
NPAIR = 12        # act pair slots: qb0 -> 4 (ktiles 0-7), qb1 -> 8 (0-15)
MASK_BIG = 1.0e12

_compiled = None


def _build(repeat=1):
    nc = bacc.Bacc("TRN2", target_bir_lowering=False, debug=False,
                   num_devices=NCORES)
    d_q = nc.dram_tensor("qT8", [D, 1024], FP8, kind="ExternalInput")
    d_k = nc.dram_tensor("kT8", [D, S], FP8, kind="ExternalInput")
    d_v = nc.dram_tensor("vT", [D, S], BF16, kind="ExternalInput")
    d_wq = nc.dram_tensor("wq8", [D, D], FP8, kind="ExternalInput")
    d_wk = nc.dram_tensor("wk8", [D, D], FP8, kind="ExternalInput")
    d_wv8h = nc.dram_tensor("wv8h", [D, D], FP8, kind="ExternalInput")
    d_wv8r = nc.dram_tensor("wv8r", [D, D], FP8, kind="ExternalInput")
    d_wo = nc.dram_tensor("wo_bf", [D, D], BF16, kind="ExternalInput")
    d_masks = nc.dram_tensor("masks", [128, 4 * 1024], BF16, kind="ExternalInput")
    d_ebias = nc.dram_tensor("ebias", [128, NPAIR], F32, kind="ExternalInput")
    d_vmcol = nc.dram_tensor("vmcol", [128, NKT], F32, kind="ExternalInput")
    d_vmcols = nc.dram_tensor("vmcols", [128, NKT], F32, kind="ExternalInput")
    d_vmaug = nc.dram_tensor("vmaug", [128, NKT * H], BF16, kind="ExternalInput")
    d_wfix = nc.dram_tensor("wfix", [128, 1024], F32, kind="ExternalInput")
    d_rvec = nc.dram_tensor("rvec", [1, 1024], F32, kind="ExternalInput")
    d_qscale = nc.dram_tensor("qscale", [1, 1024], F32, kind="ExternalInput")
    d_o = nc.dram_tensor("o", [D, 1024], F32, kind="ExternalOutput")  # o^T

    with tile.TileContext(nc) as tc, ExitStack() as ctx:
        consts = ctx.enter_context(tc.tile_pool(name="consts", bufs=1))
        big = ctx.enter_context(tc.tile_pool(name="big", bufs=1))
        stg = ctx.enter_context(tc.tile_pool(name="stg", bufs=2))
        work = ctx.enter_context(tc.tile_pool(name="work", bufs=4))
        worksm = ctx.enter_context(tc.tile_pool(name="worksm", bufs=2))
        ps_pair = ctx.enter_context(
            tc.tile_pool(name="ps_pair", bufs=2, space="PSUM"))
        ps_proj = ctx.enter_context(
            tc.tile_pool(name="ps_proj", bufs=2, space="PSUM"))
        ps_av = ctx.enter_context(
            tc.tile_pool(name="ps_av", bufs=2, space="PSUM"))

        masks_sb = consts.tile([128, 4, 2, QB], BF16)
        ebias_sb = consts.tile([128, NPAIR], F32)
        vmcol_sb = consts.tile([128, NKT], F32R)   # tvwm stationary
        vmcol_bf = consts.tile([128, NKT], BF16)   # tvwm stationary (tiles 1+)
        vmcol_f = consts.tile([128, NKT], F32)     # masking scalar
        wfix_sb = consts.tile([128, 1024], F32R)
        rvec_sb = consts.tile([1, 1024], F32R)
        qscale_sb = consts.tile([1, 1024], F32)
        ones_col = consts.tile([128, H, 1], F32)

        def emit_consts():
            nc.sync.dma_start(masks_sb[:], d_masks[:, :].rearrange(
                "p (s t q) -> p s t q", s=4, t=2))
            nc.sync.dma_start(ebias_sb[:], d_ebias[:, :])

            def load_cast(dst, dram_ap, shape):
                t0 = stg.tile(shape, F32, tag="stg")
                nc.sync.dma_start(t0[:], dram_ap)
                nc.gpsimd.tensor_copy(dst[:], t0[:])

            nc.sync.dma_start(vmcol_f[:], d_vmcols[:, :])
            load_cast(vmcol_sb, d_vmcol[:, :], [128, NKT])
            load_cast(vmcol_bf, d_vmcol[:, :], [128, NKT])
            load_cast(wfix_sb, d_wfix[:, :], [128, 1024])
            load_cast(rvec_sb, d_rvec[:, :], [1, 1024])
            nc.sync.dma_start(qscale_sb[:], d_qscale[:, :])
            nc.gpsimd.memset(ones_col[:], 1.0)

        vw0_sb = big.tile([128, H, 65], F32R)           # raw v@Wv tile0 + ones
        vwm_sb = big.tile([128, NKT, H, 65], BF16)      # masked v@Wv + vm col
        tvwm_sb = big.tile([1, H, 65], F32R)            # sum_k vm*vw (+count)
        o_sb = big.tile([128, 8, 1024], BF16)           # per-head normalized o^T

        def fold_fp8(ps_block, dst, hp2, c0, width):
            tmp = stg.tile([128, 512], FP8, tag="f8", bufs=4)
            nc.vector.tensor_copy(tmp[:], ps_block)
            for h01 in range(2):
                for dkh in range(2):
                    nc.sync.dma_start(
                        dst[h01 * 64:h01 * 64 + 32, dkh, hp2, c0:c0 + width],
                        tmp[h01 * 64 + dkh * 32:h01 * 64 + dkh * 32 + 32, :])

        # ---------- phase 1: vw + tvwm (serial emission) ----------
        def emit_p1(p1p):
            wvh_sb = p1p.tile([128, 4, 2, 1024], FP8, tag="wvh", bufs=1)
            wvr_sb = p1p.tile([128, 4, 2, 1024], FP8, tag="wvr", bufs=1)
            nc.sync.dma_start(wvh_sb[:], d_wv8h[:, :].rearrange(
                "(t j p) c -> p t j c", p=128, j=2))
            nc.sync.dma_start(wvr_sb[:], d_wv8r[:, :].rearrange(
                "(t j p) c -> p t j c", p=128, j=2))

            AHEAD = 6
            vTs = {}

            def emit_transp(sch):
                vT = p1p.tile([128, 8, 128], BF16, tag="vT", bufs=3)
                nc.sync.dma_start(
                    vT[:], d_v[:, sch * 128:(sch + 1) * 128].rearrange(
                        "(t p) c -> p t c", p=128))
                vTh = p1p.tile([128, 4, 2, 128], FP8, tag="vTh",
                               bufs=AHEAD + 1)
                vTr = p1p.tile([128, 4, 2, 128], FP8, tag="vTr",
                               bufs=AHEAD + 1)
                nc.scalar.copy(vTh[:], vT[:].rearrange(
                    "p (t j) c -> p t j c", j=2))
                nc.vector.tensor_sub(vTr[:], vT[:].rearrange(
                    "p (t j) c -> p t j c", j=2), vTh[:])
                vTs[sch] = (vTh, vTr)

            for sch in range(2):
                emit_transp(sch)
            for sch in range(2, AHEAD):
                emit_transp(sch)
            emit_consts()
            # vm columns into the denominator slots of the masked V tiles
            nc.sync.dma_start(
                vwm_sb[:, :, :, 64:65],
                d_vmaug[:, :].rearrange("p (s h o) -> p s h o", h=H, o=1))
            for sch in range(NKT):
                if sch + AHEAD < NKT:
                    emit_transp(sch + AHEAD)
                vTh, vTr = vTs.pop(sch)
                for half in range(2):
                    pv = ps_pair.tile([128, 2, 512], F32, tag="pair")
                    c0, c1 = half * 512, (half + 1) * 512
                    for dtp in range(4):
                        for a, b in ((vTh, wvh_sb), (vTh, wvr_sb),
                                     (vTr, wvh_sb)):
                            nc.tensor.matmul(
                                pv[:, 0, :], a[:, dtp, :, :],
                                b[:, dtp, :, c0:c1],
                                start=(dtp == 0 and a is vTh and b is wvh_sb),
                                stop=(dtp == 3 and a is vTr),
                                perf_mode=mybir.MatmulPerfMode.DoubleRow)
                    if sch == 0:
                        nc.vector.tensor_scalar_mul(
                            vw0_sb[:, half * 8:(half + 1) * 8, 0:64],
                            pv[:, 0, :], 1.0 / 64.0)
                    nc.vector.tensor_scalar_mul(
                        vwm_sb[:, sch, half * 8:(half + 1) * 8, 0:64],
                        pv[:, 0, :], vmcol_f[:, sch:sch + 1])
                if sch == 0:
                    nc.vector.tensor_copy(vw0_sb[:, :, 64:65], ones_col[:])
            for g in range(4):
                ptv = ps_av.tile([1, 4 * 65], F32, tag="av")
                nc.tensor.matmul(ptv[:], vmcol_sb[:, 0:1],
                                 vw0_sb[:, g * 4:(g + 1) * 4, :],
                                 start=True, stop=False)
                for kt in range(1, NKT):
                    nc.tensor.matmul(ptv[:], vmcol_bf[:, kt:kt + 1],
                                     vwm_sb[:, kt, g * 4:(g + 1) * 4, :],
                                     start=False, stop=(kt == NKT - 1))
                nc.vector.tensor_copy(tvwm_sb[0:1, g * 4:(g + 1) * 4, :],
                                      ptv[:])

        # ---------- projection of one half (generator: yields per chunk) ----
        half_tiles = {}

        def proj_gen(half, p2, p2x):
            wq_h = p2.tile([128, 4, 2, 512], FP8, tag="wh")
            nc.sync.dma_start(
                wq_h[:], d_wq[:, half * 512:(half + 1) * 512].rearrange(
                    "(t j p) c -> p t j c", p=128, j=2))
            qwT = p2.tile([128, 2, 4, 1024], FP8, tag="qwT")
            qw0_bf = p2.tile([128, 4, 512], BF16, tag="qw0")
            yield
            for qb in range(2):
                xT = p2x.tile([128, 4, 2, 512], FP8, tag="xT")
                nc.sync.dma_start(
                    xT[:], d_q[:, qb * 512:(qb + 1) * 512].rearrange(
                        "(t j p) c -> p t j c", p=128, j=2))
                for hp2 in range(4):
                    pq = ps_proj.tile([128, 512], F32, tag="proj")
                    for dtp in range(4):
                        nc.tensor.matmul(
                            pq[:], wq_h[:, dtp, :, hp2 * 128:(hp2 + 1) * 128],
                            xT[:, dtp, :, :], start=(dtp == 0),
                            stop=(dtp == 3),
                            perf_mode=mybir.MatmulPerfMode.DoubleRow)
                    fold_fp8(pq[:], qwT, hp2, qb * 512, 512)
                    if qb == 0:
                        nc.scalar.copy(qw0_bf[:, hp2, :], pq[:])
                yield
            wk_h = p2.tile([128, 4, 2, 512], FP8, tag="wh")
            nc.sync.dma_start(
                wk_h[:], d_wk[:, half * 512:(half + 1) * 512].rearrange(
                    "(t j p) c -> p t j c", p=128, j=2))
            kwT = p2.tile([128, 2, 4, 2048], FP8, tag="kwT")
            kw0_bf = p2.tile([128, 4, 256], BF16, tag="kw0")
            half_tiles[half] = (qwT, kwT, qw0_bf, kw0_bf)
            yield
            for sb in range(4):
                xT = p2x.tile([128, 4, 2, 512], FP8, tag="xT")
                nc.sync.dma_start(
                    xT[:], d_k[:, sb * 512:(sb + 1) * 512].rearrange(
                        "(t j p) c -> p t j c", p=128, j=2))
                for hp2 in range(4):
                    pk = ps_proj.tile([128, 512], F32, tag="proj")
                    for dtp in range(4):
                        nc.tensor.matmul(
                            pk[:], wk_h[:, dtp, :, hp2 * 128:(hp2 + 1) * 128],
                            xT[:, dtp, :, :], start=(dtp == 0),
                            stop=(dtp == 3),
                            perf_mode=mybir.MatmulPerfMode.DoubleRow)
                    fold_fp8(pk[:], kwT, hp2, sb * 512, 512)
                    if sb == 0:
                        nc.scalar.copy(kw0_bf[:, hp2, :], pk[:, 0:256])
                    yield

        # ---------- attention of one half (generator: yields per (h,qb)) ----
        def attn_gen(half):
            qwT, kwT, qw0_bf, kw0_bf = half_tiles[half]
            for qb in range(2):
                n_pair = 4 if qb == 0 else 8
                slot0 = 0 if qb == 0 else 4
                for hp2 in range(4):
                    for h01 in range(2):
                        h = half * 8 + hp2 * 2 + h01
                        r0 = h01 * 64
                        av = ps_av.tile([65, 512], F32, tag="av")
                        for pr in range(n_pair):
                            sc2 = ps_pair.tile([128, 2, 512], F32, tag="pair")
                            for j in range(2):
                                p = 2 * pr + j
                                if qb == 0 and pr == 0:
                                    # early causal windows (tiny softmax
                                    # support): clean bf16 scores
                                    nc.tensor.matmul(
                                        sc2[:, j, :],
                                        kw0_bf[r0:r0 + 64, hp2,
                                               j * 128:(j + 1) * 128],
                                        qw0_bf[r0:r0 + 64, hp2, :],
                                        start=True, stop=True)
                                else:
                                    nc.tensor.matmul(
                                        sc2[:, j, :],
                                        kwT[r0:r0 + 32, :, hp2,
                                            p * 128:(p + 1) * 128],
                                        qwT[r0:r0 + 32, :, hp2,
                                            qb * 512:(qb + 1) * 512],
                                        start=True, stop=True,
                                        perf_mode=
                                        mybir.MatmulPerfMode.DoubleRow)
                            et2 = work.tile([128, 2, 512], BF16, tag="et",
                                            bufs=6)
                            nc.scalar.activation(
                                et2[:], sc2[:], AF.Exp,
                                bias=ebias_sb[:, slot0 + pr:slot0 + pr + 1],
                                scale=0.125 / 4096.0)
                            if qb == 0 or pr >= 4:
                                ms = pr if qb == 0 else (pr - 2) % 4
                                nc.vector.tensor_mul(
                                    et2[:], et2[:], masks_sb[:, ms, :, :])
                            for j in range(2):
                                p = 2 * pr + j
                                nc.tensor.matmul(
                                    av[:], vwm_sb[:, p, h, :], et2[:, j, :],
                                    start=(pr == 0 and j == 0),
                                    stop=(pr == n_pair - 1 and j == 1))
                            if pr == 0 and qb == 0:
                                nc.tensor.matmul(
                                    av[:], vw0_sb[:, h, :],
                                    wfix_sb[:, 0:512],
                                    start=False, stop=False)
                                nc.tensor.matmul(
                                    av[:], tvwm_sb[0:1, h, :],
                                    rvec_sb[0:1, 0:512],
                                    start=False, stop=False)
                        recip = worksm.tile([1, 512], F32, tag="recip")
                        nc.vector.reciprocal(recip[:], av[64:65, :])
                        srow = worksm.tile([1, 512], F32R, tag="srow")
                        nc.vector.tensor_mul(
                            srow[:], recip[:],
                            qscale_sb[0:1, qb * 512:(qb + 1) * 512])
                        bsb = worksm.tile([64, 512], F32R, tag="bsb")
                        nc.gpsimd.partition_broadcast(
                            bsb[:], srow[:], channels=64)
                        nc.vector.tensor_mul(
                            o_sb[r0:r0 + 64, half * 4 + hp2,
                                 qb * 512:(qb + 1) * 512],
                            av[0:64, :], bsb[:])
                        yield

        p3_state = {}

        def emit_wo_load(p3p):
            wo_sb = p3p.tile([128, 8, 1024], BF16, tag="wo", bufs=1)
            nc.sync.dma_start(
                wo_sb[:], d_wo[:, :].rearrange("(t p) c -> p t c", p=128))
            p3_state["wo"] = wo_sb

        def p3_qb_gen(p3p, qb):
            wo_sb = p3_state["wo"]
            for oc in range(8):
                po = ps_proj.tile([128, 512], F32, tag="proj")
                for hp in range(8):
                    nc.tensor.matmul(
                        po[:], wo_sb[:, hp, oc * 128:(oc + 1) * 128],
                        o_sb[:, hp, qb * 512:(qb + 1) * 512],
                        start=(hp == 0), stop=(hp == 7))
                ot = p3p.tile([128, 512], F32, tag="ot")
                nc.vector.tensor_copy(ot[:], po[:])
                nc.sync.dma_start(
                    d_o[oc * 128:(oc + 1) * 128,
                        qb * 512:(qb + 1) * 512], ot[:])
                yield

        for _rep in range(repeat):
            with tc.tile_pool(name="p1p", bufs=2) as p1p:
                emit_p1(p1p)
            with tc.tile_pool(name="p2", bufs=2) as p2, \
                 tc.tile_pool(name="p2x", bufs=2) as p2x, \
                 tc.tile_pool(name="p3p", bufs=2) as p3p:
                g = proj_gen(0, p2, p2x)
                for _ in g:
                    pass
                g_next = proj_gen(1, p2, p2x)
                for i, _ in enumerate(attn_gen(0)):
                    next(g_next, None)
                for _ in g_next:
                    pass
                emit_wo_load(p3p)
                g3 = None
                for i, _ in enumerate(attn_gen(1)):
                    if i == 8:
                        g3 = p3_qb_gen(p3p, 0)
                    if g3 is not None:
                        next(g3, None)
                if g3 is not None:
                    for _ in g3:
                        pass
                for _ in p3_qb_gen(p3p, 1):
                    pass

    nc.compile()
    return nc


def _host_data(q, k, v, q_mask, v_mask, Wq, Wk, Wv, Wo):
    """Build the 8 per-core input maps."""
    ki = np.arange(128)[:, None]
    qi = np.arange(QB)[None, :]
    tri = [(qi >= ki + j * 128).astype(ml_dtypes.bfloat16) for j in range(4)]
    ones_m = np.ones((128, QB), ml_dtypes.bfloat16)
    tri_pairs = [np.concatenate([tri[0], tri[1]], 1),
                 np.concatenate([tri[2], tri[3]], 1)]
    ones_pair = np.concatenate([ones_m, ones_m], 1)

    wq8 = np.ascontiguousarray((Wq * 64.0).astype(ml_dtypes.float8_e4m3))
    wv8h = np.ascontiguousarray((Wv * 64.0).astype(ml_dtypes.float8_e4m3))
    wv8r = np.ascontiguousarray(
        (Wv * 64.0 - wv8h.astype(np.float32)).astype(ml_dtypes.float8_e4m3))
    wo_bf = np.ascontiguousarray(Wo.astype(ml_dtypes.bfloat16))
    wk8 = np.ascontiguousarray((Wk * 64.0).astype(ml_dtypes.float8_e4m3))
    kT8_all = [np.ascontiguousarray(k[b].T.astype(ml_dtypes.float8_e4m3))
               for b in range(B)]
    vT_all = [np.ascontiguousarray(v[b].T.astype(ml_dtypes.bfloat16))
              for b in range(B)]
    in_maps = []
    for c in range(NCORES):
        b, qh = c // 2, c % 2
        kT8 = kT8_all[b]
        if qh == 0:
            chunks = (0, 3)
            # storage (A,B,C,D); qb0 reads (A,B,C,D), qb1 pairs 4-7 read
            # (C,D,A,B) — covers both cores' tri/ones arrangements.
            slots = [tri_pairs[0], tri_pairs[1], ones_pair, ones_pair]
            ebias_cols = [2, 3]     # qb0 dummy pairs (ktiles 4-7)
        else:
            chunks = (1, 2)
            slots = [ones_pair, ones_pair, tri_pairs[0], tri_pairs[1]]
            ebias_cols = [10, 11]   # qb1 dummy pairs (ktiles 12-15)
        rows = np.r_[chunks[0] * 512:(chunks[0] + 1) * 512,
                     chunks[1] * 512:(chunks[1] + 1) * 512]

        vm = v_mask[b].astype(np.float32)
        qm = q_mask[b].astype(np.float32)
        ebias = np.zeros((128, NPAIR), np.float32)
        for col in ebias_cols:
            ebias[:, col] = -MASK_BIG

        vmcol = np.ascontiguousarray(vm.reshape(NKT, 128).T)
        vmaug = np.broadcast_to(
            vmcol.astype(ml_dtypes.bfloat16)[:, :, None],
            (128, NKT, H)).reshape(128, NKT * H)

        # fully-masked-row fix
        r = (np.cumsum(vm) == 0).astype(np.float32)
        fix_rows = np.where(r > 0)[0]
        assert fix_rows.size == 0 or fix_rows.max() < 128, \
            "fully-masked query rows beyond 128 unsupported"
        wfix = np.zeros((128, 1024), np.float32)
        rvec = np.zeros((1, 1024), np.float32)
        if qh == 0 and fix_rows.size:
            for qq in fix_rows:            # local row == global row (< 512)
                wfix[:qq + 1, qq] = (1.0 - vm[:128])[:qq + 1]
            rvec[0, :128] = r[:128]

        in_maps.append({
            "qT8": np.ascontiguousarray(
                q[b][rows].T.astype(ml_dtypes.float8_e4m3)),
            "kT8": kT8,
            "vT": vT_all[b],
            "wq8": wq8, "wk8": wk8, "wv8h": wv8h, "wv8r": wv8r,
            "wo_bf": wo_bf,
            "masks": np.ascontiguousarray(
                np.stack(slots, 1).reshape(128, 4 * 1024)),
            "ebias": ebias,
            "vmcol": vmcol,
            "vmcols": np.ascontiguousarray(vmcol / 64.0),
            "vmaug": np.ascontiguousarray(vmaug),
            "wfix": wfix,
            "rvec": rvec,
            "qscale": np.ascontiguousarray(qm[rows].reshape(1, 1024)),
        })
    return in_maps


def kernel(q, k, v, q_mask, v_mask, Wq, bq, Wk, bk, Wv, bv, Wo, bo,
           **run_kwargs):
    global _compiled
    q = np.asarray(q, np.float32)
    k = np.asarray(k, np.float32)
    v = np.asarray(v, np.float32)
    q_mask = np.asarray(q_mask)
    v_mask = np.asarray(v_mask)
    assert q.shape == (B, S, D)
    # biases are structurally zero in this problem
    for bias in (bq, bk, bv, bo):
        assert np.all(np.asarray(bias) == 0.0)

    if _compiled is None:
        _compiled = _build()
    in_maps = _host_data(q, k, v, q_mask, v_mask,
                         np.ascontiguousarray(np.asarray(Wq, np.float32)),
                         np.ascontiguousarray(np.asarray(Wk, np.float32)),
                         np.ascontiguousarray(np.asarray(Wv, np.float32)),
                         np.ascontiguousarray(np.asarray(Wo, np.float32)))
    # The device occasionally returns silently-corrupted results after a
    # transient fault; run twice and retry until two runs agree.
    res = bass_utils.run_bass_kernel_spmd(
        _compiled, in_maps, core_ids=list(range(NCORES)), **run_kwargs)
    for _attempt in range(3):
        res2 = bass_utils.run_bass_kernel_spmd(
            _compiled, in_maps, core_ids=list(range(NCORES)), **run_kwargs)
        diff = max(
            float(np.max(np.abs(res.results[c]["o"] - res2.results[c]["o"])))
            for c in range(NCORES))
        if diff < 1e-3:
            break
        res = res2

    out = np.empty((B, S, D), np.float32)
    for c in range(NCORES):
        b, qh = c // 2, c % 2
        chunks = (0, 3) if qh == 0 else (1, 2)
        oT = res.results[c]["o"]            # [D, 1024]
        out[b, chunks[0] * 512:(chunks[0] + 1) * 512] = oT[:, 0:512].T
        out[b, chunks[1] * 512:(chunks[1] + 1) * 512] = oT[:, 512:1024].T
    if run_kwargs:
        kernel.last_results = res
    return out


# revision 39
# speedup vs baseline: 1.4548x; 1.4548x over previous
"""Multi-head attention (B=4, S=2048, D=1024, H=16, dk=dv=64, causal + key/query
masks) on 8 Trainium2 NeuronCores via Bass/Tile.

Sharding: core = (batch, query-half). Each core handles one batch's 1024 query
rows (two 512-row chunks paired across the causal diagonal: {0,3} or {1,2}) and
redundantly computes K/V projections for its batch — no collectives.

v2 structure (vs baseline): the attention inner loop processes k-tile PAIRS —
scores for two 128-key tiles land in one [128,2,512] PSUM tile and a single
activation computes exp over 1024 free elements, halving ACT-engine instruction
count and amortizing its fixed access latency. v_mask is folded into the V
tiles at projection-copy time (rows zeroed, denominator column = vm), so the
activation needs no per-key bias; a per-pair bias (-1e12) zeroes the dummy
pairs that pad the causal structure. Fully-masked query rows (reference
softmax = uniform over the exactly -1e12 entries) are reproduced with the
raw (unmasked) k-tile-0 V tile plus two accumulating fix matmuls, emitted only
for the qb=0 slot (fix rows are always < 128). All PSUM->SBUF copies run on
DVE/Pool so the ACT engine does exp exclusively.
"""

import os

os.environ.setdefault("JAX_PLATFORMS", "axon,cpu")

import numpy as np
import ml_dtypes
from contextlib import ExitStack

import concourse.bass as bass
import concourse.mybir as mybir
import concourse.tile as tile
from concourse import bacc
from concourse import bass_utils

F32 = mybir.dt.float32
FP8 = mybir.dt.float8e4
F32R = mybir.dt.float32r
BF16 = mybir.dt.bfloat16
AF = mybir.ActivationFunctionType

B, S, D = 4, 2048, 1024
H, DK, DV = 16, 64, 64
NCORES = 8
QB = 512          # query block (matmul moving dim)
KT = 128          # key tile (scores partition dim)
NKT = S // KT     # 16
NPAIR = 12        # act pair slots: qb0 -> 4 (ktiles 0-7), qb1 -> 8 (0-15)
MASK_BIG = 1.0e12

_compiled = None


def _build(repeat=1):
    nc = bacc.Bacc("TRN2", target_bir_lowering=False, debug=False,
                   num_devices=NCORES)
    d_q = nc.dram_tensor("qT8", [D, 1024], FP8, kind="ExternalInput")
    d_k = nc.dram_tensor("kT8", [D, S], FP8, kind="ExternalInput")
    d_v = nc.dram_tensor("vT", [D, S], BF16, kind="ExternalInput")
    d_wq = nc.dram_tensor("wq8", [D, D], FP8, kind="ExternalInput")
    d_wk = nc.dram_tensor("wk8", [D, D], FP8, kind="ExternalInput")
    d_wv8h = nc.dram_tensor("wv8h", [D, D], FP8, kind="ExternalInput")
    d_wv8r = nc.dram_tensor("wv8r", [D, D], FP8, kind="ExternalInput")
    d_wo = nc.dram_tensor("wo_bf", [D, D], BF16, kind="ExternalInput")
    d_masks = nc.dram_tensor("masks", [128, 4 * 1024], BF16, kind="ExternalInput")
    d_ebias = nc.dram_tensor("ebias", [128, NPAIR], F32, kind="ExternalInput")
    d_vmcol = nc.dram_tensor("vmcol", [128, NKT], F32, kind="ExternalInput")
    d_vmcols = nc.dram_tensor("vmcols", [128, NKT], F32, kind="ExternalInput")
    d_vmaug = nc.dram_tensor("vmaug", [128, NKT * H], BF16, kind="ExternalInput")
    d_wfix = nc.dram_tensor("wfix", [128, 1024], F32, kind="ExternalInput")
    d_rvec = nc.dram_tensor("rvec", [1, 1024], F32, kind="ExternalInput")
    d_qscale = nc.dram_tensor("qscale", [1, 1024], F32, kind="ExternalInput")
    d_o = nc.dram_tensor("o", [D, 1024], F32, kind="ExternalOutput")  # o^T

    with tile.TileContext(nc) as tc, ExitStack() as ctx:
        consts = ctx.enter_context(tc.tile_pool(name="consts", bufs=1))
        big = ctx.enter_context(tc.tile_pool(name="big", bufs=1))
        stg = ctx.enter_context(tc.tile_pool(name="stg", bufs=2))
        work = ctx.enter_context(tc.tile_pool(name="work", bufs=4))
        worksm = ctx.enter_context(tc.tile_pool(name="worksm", bufs=2))
        ps_pair = ctx.enter_context(
            tc.tile_pool(name="ps_pair", bufs=2, space="PSUM"))
        ps_proj = ctx.enter_context(
            tc.tile_pool(name="ps_proj", bufs=2, space="PSUM"))
        ps_av = ctx.enter_context(
            tc.tile_pool(name="ps_av", bufs=2, space="PSUM"))

        masks_sb = consts.tile([128, 4, 2, QB], BF16)
        ebias_sb = consts.tile([128, NPAIR], F32)
        vmcol_sb = consts.tile([128, NKT], F32R)   # tvwm stationary
        vmcol_bf = consts.tile([128, NKT], BF16)   # tvwm stationary (tiles 1+)
        vmcol_f = consts.tile([128, NKT], F32)     # masking scalar
        wfix_sb = consts.tile([128, 1024], F32R)
        rvec_sb = consts.tile([1, 1024], F32R)
        qscale_sb = consts.tile([1, 1024], F32)
        ones_col = consts.tile([128, H, 1], F32)

        def emit_consts():
            nc.sync.dma_start(masks_sb[:], d_masks[:, :].rearrange(
                "p (s t q) -> p s t q", s=4, t=2))
            nc.sync.dma_start(ebias_sb[:], d_ebias[:, :])

            def load_cast(dst, dram_ap, shape):
                t0 = stg.tile(shape, F32, tag="stg")
                nc.sync.dma_start(t0[:], dram_ap)
                nc.gpsimd.tensor_copy(dst[:], t0[:])

            nc.sync.dma_start(vmcol_f[:], d_vmcols[:, :])
            load_cast(vmcol_sb, d_vmcol[:, :], [128, NKT])
            load_cast(vmcol_bf, d_vmcol[:, :], [128, NKT])
            load_cast(wfix_sb, d_wfix[:, :], [128, 1024])
            load_cast(rvec_sb, d_rvec[:, :], [1, 1024])
            nc.sync.dma_start(qscale_sb[:], d_qscale[:, :])
            nc.gpsimd.memset(ones_col[:], 1.0)

        vw0_sb = big.tile([128, H, 65], F32R)           # raw v@Wv tile0 + ones
        vwm_sb = big.tile([128, NKT, H, 65], BF16)      # masked v@Wv + vm col
        tvwm_sb = big.tile([1, H, 65], F32R)            # sum_k vm*vw (+count)
        o_sb = big.tile([128, 8, 1024], BF16)           # per-head normalized o^T

        def fold_fp8(ps_block, dst, hp2, c0, width):
            tmp = stg.tile([128, 512], FP8, tag="f8")
            nc.vector.tensor_copy(tmp[:], ps_block)
            for h01 in range(2):
                for dkh in range(2):
                    nc.sync.dma_start(
                        dst[h01 * 64:h01 * 64 + 32, dkh, hp2, c0:c0 + width],
                        tmp[h01 * 64 + dkh * 32:h01 * 64 + dkh * 32 + 32, :])

        # ---------- phase 1: vw + tvwm (serial emission) ----------
        def emit_p1(p1p):
            wvh_sb = p1p.tile([128, 4, 2, 1024], FP8, tag="wvh", bufs=1)
            wvr_sb = p1p.tile([128, 4, 2, 1024], FP8, tag="wvr", bufs=1)
            nc.sync.dma_start(wvh_sb[:], d_wv8h[:, :].rearrange(
                "(t j p) c -> p t j c", p=128, j=2))
            nc.sync.dma_start(wvr_sb[:], d_wv8r[:, :].rearrange(
                "(t j p) c -> p t j c", p=128, j=2))

            AHEAD = 6
            vTs = {}

            def emit_transp(sch):
                vT = p1p.tile([128, 8, 128], BF16, tag="vT", bufs=3)
                nc.sync.dma_start(
                    vT[:], d_v[:, sch * 128:(sch + 1) * 128].rearrange(
                        "(t p) c -> p t c", p=128))
                vTh = p1p.tile([128, 4, 2, 128], FP8, tag="vTh",
                               bufs=AHEAD + 1)
                vTr = p1p.tile([128, 4, 2, 128], FP8, tag="vTr",
                               bufs=AHEAD + 1)
                nc.scalar.copy(vTh[:], vT[:].rearrange(
                    "p (t j) c -> p t j c", j=2))
                nc.vector.tensor_sub(vTr[:], vT[:].rearrange(
                    "p (t j) c -> p t j c", j=2), vTh[:])
                vTs[sch] = (vTh, vTr)

            for sch in range(2):
                emit_transp(sch)
            for sch in range(2, AHEAD):
                emit_transp(sch)
            emit_consts()
            # vm columns into the denominator slots of the masked V tiles
            nc.sync.dma_start(
                vwm_sb[:, :, :, 64:65],
                d_vmaug[:, :].rearrange("p (s h o) -> p s h o", h=H, o=1))
            for sch in range(NKT):
                if sch + AHEAD < NKT:
                    emit_transp(sch + AHEAD)
                vTh, vTr = vTs.pop(sch)
                for half in range(2):
                    pv = ps_pair.tile([128, 2, 512], F32, tag="pair")
                    c0, c1 = half * 512, (half + 1) * 512
                    for dtp in range(4):
                        for a, b in ((vTh, wvh_sb), (vTh, wvr_sb),
                                     (vTr, wvh_sb)):
                            nc.tensor.matmul(
                                pv[:, 0, :], a[:, dtp, :, :],
                                b[:, dtp, :, c0:c1],
                                start=(dtp == 0 and a is vTh and b is wvh_sb),
                                stop=(dtp == 3 and a is vTr),
                                perf_mode=mybir.MatmulPerfMode.DoubleRow)
                    if sch == 0:
                        nc.vector.tensor_scalar_mul(
                            vw0_sb[:, half * 8:(half + 1) * 8, 0:64],
                            pv[:, 0, :], 1.0 / 64.0)
                    nc.vector.tensor_scalar_mul(
                        vwm_sb[:, sch, half * 8:(half + 1) * 8, 0:64],
                        pv[:, 0, :], vmcol_f[:, sch:sch + 1])
                if sch == 0:
                    nc.vector.tensor_copy(vw0_sb[:, :, 64:65], ones_col[:])
            for g in range(4):
                ptv = ps_av.tile([1, 4 * 65], F32, tag="av")
                nc.tensor.matmul(ptv[:], vmcol_sb[:, 0:1],
                                 vw0_sb[:, g * 4:(g + 1) * 4, :],
                                 start=True, stop=False)
                for kt in range(1, NKT):
                    nc.tensor.matmul(ptv[:], vmcol_bf[:, kt:kt + 1],
                                     vwm_sb[:, kt, g * 4:(g + 1) * 4, :],
                                     start=False, stop=(kt == NKT - 1))
                nc.vector.tensor_copy(tvwm_sb[0:1, g * 4:(g + 1) * 4, :],
                                      ptv[:])

        # ---------- projection of one half (generator: yields per chunk) ----
        half_tiles = {}

        def proj_gen(half, p2, p2x):
            wq_h = p2.tile([128, 4, 2, 512], FP8, tag="wh")
            nc.sync.dma_start(
                wq_h[:], d_wq[:, half * 512:(half + 1) * 512].rearrange(
                    "(t j p) c -> p t j c", p=128, j=2))
            qwT = p2.tile([128, 2, 4, 1024], FP8, tag="qwT")
            qw0_bf = p2.tile([128, 4, 512], BF16, tag="qw0")
            yield
            for qb in range(2):
                xT = p2x.tile([128, 4, 2, 512], FP8, tag="xT")
                nc.sync.dma_start(
                    xT[:], d_q[:, qb * 512:(qb + 1) * 512].rearrange(
                        "(t j p) c -> p t j c", p=128, j=2))
                for hp2 in range(4):
                    pq = ps_proj.tile([128, 512], F32, tag="proj")
                    for dtp in range(4):
                        nc.tensor.matmul(
                            pq[:], wq_h[:, dtp, :, hp2 * 128:(hp2 + 1) * 128],
                            xT[:, dtp, :, :], start=(dtp == 0),
                            stop=(dtp == 3),
                            perf_mode=mybir.MatmulPerfMode.DoubleRow)
                    fold_fp8(pq[:], qwT, hp2, qb * 512, 512)
                    if qb == 0:
                        nc.scalar.copy(qw0_bf[:, hp2, :], pq[:])
                yield
            wk_h = p2.tile([128, 4, 2, 512], FP8, tag="wh")
            nc.sync.dma_start(
                wk_h[:], d_wk[:, half * 512:(half + 1) * 512].rearrange(
                    "(t j p) c -> p t j c", p=128, j=2))
            kwT = p2.tile([128, 2, 4, 2048], FP8, tag="kwT")
            kw0_bf = p2.tile([128, 4, 256], BF16, tag="kw0")
            half_tiles[half] = (qwT, kwT, qw0_bf, kw0_bf)
            yield
            for sb in range(4):
                xT = p2x.tile([128, 4, 2, 512], FP8, tag="xT")
                nc.sync.dma_start(
                    xT[:], d_k[:, sb * 512:(sb + 1) * 512].rearrange(
                        "(t j p) c -> p t j c", p=128, j=2))
                for hp2 in range(4):
                    pk = ps_proj.tile([128, 512], F32, tag="proj")
                    for dtp in range(4):
                        nc.tensor.matmul(
                            pk[:], wk_h[:, dtp, :, hp2 * 128:(hp2 + 1) * 128],
                            xT[:, dtp, :, :], start=(dtp == 0),
                            stop=(dtp == 3),
                            perf_mode=mybir.MatmulPerfMode.DoubleRow)
                    fold_fp8(pk[:], kwT, hp2, sb * 512, 512)
                    if sb == 0:
                        nc.scalar.copy(kw0_bf[:, hp2, :], pk[:, 0:256])
                    yield

        # ---------- attention of one half (generator: yields per (h,qb)) ----
        def attn_gen(half):
            qwT, kwT, qw0_bf, kw0_bf = half_tiles[half]
            for qb in range(2):
                n_pair = 4 if qb == 0 else 8
                slot0 = 0 if qb == 0 else 4
                for hp2 in range(4):
                    for h01 in range(2):
                        h = half * 8 + hp2 * 2 + h01
                        r0 = h01 * 64
                        av = ps_av.tile([65, 512], F32, tag="av")
                        for pr in range(n_pair):
                            sc2 = ps_pair.tile([128, 2, 512], F32, tag="pair")
                            for j in range(2):
                                p = 2 * pr + j
                                if qb == 0 and pr == 0:
                                    # early causal windows (tiny softmax
                                    # support): clean bf16 scores
                                    nc.tensor.matmul(
                                        sc2[:, j, :],
                                        kw0_bf[r0:r0 + 64, hp2,
                                               j * 128:(j + 1) * 128],
                                        qw0_bf[r0:r0 + 64, hp2, :],
                                        start=True, stop=True)
                                else:
                                    nc.tensor.matmul(
                                        sc2[:, j, :],
                                        kwT[r0:r0 + 32, :, hp2,
                                            p * 128:(p + 1) * 128],
                                        qwT[r0:r0 + 32, :, hp2,
                                            qb * 512:(qb + 1) * 512],
                                        start=True, stop=True,
                                        perf_mode=
                                        mybir.MatmulPerfMode.DoubleRow)
                            et2 = work.tile([128, 2, 512], BF16, tag="et",
                                            bufs=6)
                            nc.scalar.activation(
                                et2[:], sc2[:], AF.Exp,
                                bias=ebias_sb[:, slot0 + pr:slot0 + pr + 1],
                                scale=0.125 / 4096.0)
                            if qb == 0 or pr >= 4:
                                ms = pr if qb == 0 else (pr - 2) % 4
                                nc.vector.tensor_mul(
                                    et2[:], et2[:], masks_sb[:, ms, :, :])
                            for j in range(2):
                                p = 2 * pr + j
                                nc.tensor.matmul(
                                    av[:], vwm_sb[:, p, h, :], et2[:, j, :],
                                    start=(pr == 0 and j == 0),
                                    stop=(pr == n_pair - 1 and j == 1))
                            if pr == 0 and qb == 0:
                                nc.tensor.matmul(
                                    av[:], vw0_sb[:, h, :],
                                    wfix_sb[:, 0:512],
                                    start=False, stop=False)
                                nc.tensor.matmul(
                                    av[:], tvwm_sb[0:1, h, :],
                                    rvec_sb[0:1, 0:512],
                                    start=False, stop=False)
                        recip = worksm.tile([1, 512], F32, tag="recip")
                        nc.vector.reciprocal(recip[:], av[64:65, :])
                        srow = worksm.tile([1, 512], F32R, tag="srow")
                        nc.vector.tensor_mul(
                            srow[:], recip[:],
                            qscale_sb[0:1, qb * 512:(qb + 1) * 512])
                        bsb = worksm.tile([64, 512], F32R, tag="bsb")
                        nc.gpsimd.partition_broadcast(
                            bsb[:], srow[:], channels=64)
                        nc.vector.tensor_mul(
                            o_sb[r0:r0 + 64, half * 4 + hp2,
                                 qb * 512:(qb + 1) * 512],
                            av[0:64, :], bsb[:])
                        yield

        p3_state = {}

        def emit_wo_load(p3p):
            wo_sb = p3p.tile([128, 8, 1024], BF16, tag="wo", bufs=1)
            nc.sync.dma_start(
                wo_sb[:], d_wo[:, :].rearrange("(t p) c -> p t c", p=128))
            p3_state["wo"] = wo_sb

        def p3_qb_gen(p3p, qb):
            wo_sb = p3_state["wo"]
            for oc in range(8):
                po = ps_proj.tile([128, 512], F32, tag="proj")
                for hp in range(8):
                    nc.tensor.matmul(
                        po[:], wo_sb[:, hp, oc * 128:(oc + 1) * 128],
                        o_sb[:, hp, qb * 512:(qb + 1) * 512],
                        start=(hp == 0), stop=(hp == 7))
                ot = p3p.tile([128, 512], F32, tag="ot")
                nc.vector.tensor_copy(ot[:], po[:])
                nc.sync.dma_start(
                    d_o[oc * 128:(oc + 1) * 128,
                        qb * 512:(qb + 1) * 512], ot[:])
                yield

        for _rep in range(repeat):
            with tc.tile_pool(name="p1p", bufs=2) as p1p:
                emit_p1(p1p)
            with tc.tile_pool(name="p2", bufs=2) as p2, \
                 tc.tile_pool(name="p2x", bufs=2) as p2x, \
                 tc.tile_pool(name="p3p", bufs=2) as p3p:
                g = proj_gen(0, p2, p2x)
                for _ in g:
                    pass
                g_next = proj_gen(1, p2, p2x)
                for i, _ in enumerate(attn_gen(0)):
                    next(g_next, None)
                for _ in g_next:
                    pass
                emit_wo_load(p3p)
                g3 = None
                for i, _ in enumerate(attn_gen(1)):
                    if i == 8:
                        g3 = p3_qb_gen(p3p, 0)
                    if g3 is not None:
                        next(g3, None)
                if g3 is not None:
                    for _ in g3:
                        pass
                for _ in p3_qb_gen(p3p, 1):
                    pass

    nc.compile()
    return nc


def _host_data(q, k, v, q_mask, v_mask, Wq, Wk, Wv, Wo):
    """Build the 8 per-core input maps."""
    ki = np.arange(128)[:, None]
    qi = np.arange(QB)[None, :]
    tri = [(qi >= ki + j * 128).astype(ml_dtypes.bfloat16) for j in range(4)]
    ones_m = np.ones((128, QB), ml_dtypes.bfloat16)
    tri_pairs = [np.concatenate([tri[0], tri[1]], 1),
                 np.concatenate([tri[2], tri[3]], 1)]
    ones_pair = np.concatenate([ones_m, ones_m], 1)

    wq8 = np.ascontiguousarray((Wq * 64.0).astype(ml_dtypes.float8_e4m3))
    wv8h = np.ascontiguousarray((Wv * 64.0).astype(ml_dtypes.float8_e4m3))
    wv8r = np.ascontiguousarray(
        (Wv * 64.0 - wv8h.astype(np.float32)).astype(ml_dtypes.float8_e4m3))
    wo_bf = np.ascontiguousarray(Wo.astype(ml_dtypes.bfloat16))
    wk8 = np.ascontiguousarray((Wk * 64.0).astype(ml_dtypes.float8_e4m3))
    kT8_all = [np.ascontiguousarray(k[b].T.astype(ml_dtypes.float8_e4m3))
               for b in range(B)]
    vT_all = [np.ascontiguousarray(v[b].T.astype(ml_dtypes.bfloat16))
              for b in range(B)]
    in_maps = []
    for c in range(NCORES):
        b, qh = c // 2, c % 2
        kT8 = kT8_all[b]
        if qh == 0:
            chunks = (0, 3)
            # storage (A,B,C,D); qb0 reads (A,B,C,D), qb1 pairs 4-7 read
            # (C,D,A,B) — covers both cores' tri/ones arrangements.
            slots = [tri_pairs[0], tri_pairs[1], ones_pair, ones_pair]
            ebias_cols = [2, 3]     # qb0 dummy pairs (ktiles 4-7)
        else:
            chunks = (1, 2)
            slots = [ones_pair, ones_pair, tri_pairs[0], tri_pairs[1]]
            ebias_cols = [10, 11]   # qb1 dummy pairs (ktiles 12-15)
        rows = np.r_[chunks[0] * 512:(chunks[0] + 1) * 512,
                     chunks[1] * 512:(chunks[1] + 1) * 512]

        vm = v_mask[b].astype(np.float32)
        qm = q_mask[b].astype(np.float32)
        ebias = np.zeros((128, NPAIR), np.float32)
        for col in ebias_cols:
            ebias[:, col] = -MASK_BIG

        vmcol = np.ascontiguousarray(vm.reshape(NKT, 128).T)
        vmaug = np.broadcast_to(
            vmcol.astype(ml_dtypes.bfloat16)[:, :, None],
            (128, NKT, H)).reshape(128, NKT * H)

        # fully-masked-row fix
        r = (np.cumsum(vm) == 0).astype(np.float32)
        fix_rows = np.where(r > 0)[0]
        assert fix_rows.size == 0 or fix_rows.max() < 128, \
            "fully-masked query rows beyond 128 unsupported"
        wfix = np.zeros((128, 1024), np.float32)
        rvec = np.zeros((1, 1024), np.float32)
        if qh == 0 and fix_rows.size:
            for qq in fix_rows:            # local row == global row (< 512)
                wfix[:qq + 1, qq] = (1.0 - vm[:128])[:qq + 1]
            rvec[0, :128] = r[:128]

        in_maps.append({
            "qT8": np.ascontiguousarray(
                q[b][rows].T.astype(ml_dtypes.float8_e4m3)),
            "kT8": kT8,
            "vT": vT_all[b],
            "wq8": wq8, "wk8": wk8, "wv8h": wv8h, "wv8r": wv8r,
            "wo_bf": wo_bf,
            "masks": np.ascontiguousarray(
                np.stack(slots, 1).reshape(128, 4 * 1024)),
            "ebias": ebias,
            "vmcol": vmcol,
            "vmcols": np.ascontiguousarray(vmcol / 64.0),
            "vmaug": np.ascontiguousarray(vmaug),
            "wfix": wfix,
            "rvec": rvec,
            "qscale": np.ascontiguousarray(qm[rows].reshape(1, 1024)),
        })
    return in_maps


def kernel(q, k, v, q_mask, v_mask, Wq, bq, Wk, bk, Wv, bv, Wo, bo,
           **run_kwargs):
    global _compiled
    q = np.asarray(q, np.float32)
    k = np.asarray(k, np.float32)
    v = np.asarray(v, np.float32)
    q_mask = np.asarray(q_mask)
    v_mask = np.asarray(v_mask)
    assert q.shape == (B, S, D)
    # biases are structurally zero in this problem
    for bias in (bq, bk, bv, bo):
        assert np.all(np.asarray(bias) == 0.0)

    if _compiled is None:
        _compiled = _build()
    in_maps = _host_data(q, k, v, q_mask, v_mask,
                         np.ascontiguousarray(np.asarray(Wq, np.float32)),
                         np.ascontiguousarray(np.asarray(Wk, np.float32)),
                         np.ascontiguousarray(np.asarray(Wv, np.float32)),
                         np.ascontiguousarray(np.asarray(Wo, np.float32)))
    # The device occasionally returns silently-corrupted results after a
    # transient fault; run twice and retry until two runs agree.
    res = bass_utils.run_bass_kernel_spmd(
        _compiled, in_maps, core_ids=list(range(NCORES)), **run_kwargs)
    for _attempt in range(3):
        res2 = bass_utils.run_bass_kernel_spmd(
            _compiled, in_maps, core_ids=list(range(NCORES)), **run_kwargs)
        diff = max(
            float(np.max(np.abs(res.results[c]["o"] - res2.results[c]["o"])))
            for c in range(NCORES))
        if diff < 1e-3:
            break
        res = res2

    out = np.empty((B, S, D), np.float32)
    for c in range(NCORES):
        b, qh = c // 2, c % 2
        chunks = (0, 3) if qh == 0 else (1, 2)
        oT = res.results[c]["o"]            # [D, 1024]
        out[b, chunks[0] * 512:(chunks[0] + 1) * 512] = oT[:, 0:512].T
        out[b, chunks[1] * 512:(chunks[1] + 1) * 512] = oT[:, 512:1024].T
    if run_kwargs:
        kernel.last_results = res
    return out
